# revision 1
# baseline (speedup 1.0000x reference)
"""Trainium2 Bass kernel for softmax-free attention:
    q = x @ Wq^T; k = x @ Wk^T; v = x @ Wv^T
    s = (q @ k^T) / sqrt(d); out = s @ v
  x: [4, 4096, 1024], W*: [1024, 1024], out: [4, 4096, 1024] (fp32)

Sharding: 8 cores; core c handles batch c//2, sequence-half c%2 (2048 query
rows). Each core projects q/k/v only for its OWN 2048 rows and spills k/v
into a cross-core-visible Shared-DRAM buffer (slot = own rank-in-pair via a
dynamic DMA offset). The pair partner reads both halves at local HBM
bandwidth — no bulk collective. Ordering across the pair is a tiny token
AllReduce (the token is DMA-sampled from the shared buffers, so it carries a
RAW dep on all spill writes); only the peer-slot reads wait on it, and they
start ~120us after it fires. The per-core x input is column-ROTATED on the
host (own half first); attention is permutation-invariant over m as long as
k and v use the same order.

Layout strategy: the PE contracts over the partition dim, so every operand is
arranged K-on-partitions via host-side pre-transposes (xT = x[b].T, W^T) and
chained matmuls that produce transposed outputs directly:
  qT[e,l] = sum_d WqT[d,e] xT[d,l]     (lhsT=WqT chunk, rhs=xT chunk)
  kT[e,m] = likewise
  v[m,d'] = sum_d xT[d,m] WvT[d,d']    (lhsT=xT chunk,  rhs=WvT chunk)
  sT[m,l] = sum_e kT[e,m] qT[e,l]      (lhsT=kT chunk,  rhs=qT chunk)
  out[l,d']= sum_m sT[m,l] v[m,d']     (lhsT=sT chunk,  rhs=v chunk)
The 1/sqrt(d) scale is folded into WqT on the host. All matmul inputs are
float32r (full PE rate at free-dim>=256, ~1e-4 rel err).

Phase A streams the own xT half once, producing kT + v (spilled to shared
DRAM) and qT (kept resident in SBUF). Phase B processes the 2048 query rows in two 1024-row
blocks, streaming kT/v back in 512-row m-chunks (4-matmul PSUM accumulation
groups keep the PE efficient) and accumulating out in SBUF via DVE adds.
"""

import sys
import types
from contextlib import ExitStack

import numpy as np

import concourse.bass as bass
import concourse.tile as tile
from concourse import bacc, mybir
from concourse.bass_utils import run_bass_kernel_spmd
from concourse.mybir import EngineType
from concourse.tile import add_dep_helper
from concourse.vector_clock import ScopedClock

# ---------------------------------------------------------------------------
# Environment shims
# ---------------------------------------------------------------------------


def _install_tile_drain_patch():
    """This toolchain's walrus caps sync waits at 1 per instruction, but
    TileContext's tail drain can carry several. Split the overflow onto
    preceding nops (same semantics: the issuing engine observes every sem
    before draining)."""
    if getattr(tile.TileContext, "_drain_patch_installed", False):
        return

    def _patched_drain_and_barrier(self, tick_clock, wait_clock):
        nc = self.nc
        collector = nc.sync.nop(hint="drain_wait_collector", nofuse=True)
        wait_clock.add_sem_waits(
            collector.ins, ScopedClock({None: tick_clock.global_clock})
        )
        waits = list(collector.ins.sync_info.on_wait or [])
        if len(waits) > 1:
            collector.ins.sync_info.on_wait = [waits[0]]
            for w in waits[1:]:
                nop = nc.sync.nop(hint="drain_wait_extra", nofuse=True)
                nop.ins.sync_info = mybir.SyncInfo(on_wait=[w], on_update=[])
        nc.sync.drain()

        nc.all_engine_barrier()
        assert self.sems is not None
        popped = nc._tile_sem_poison_stack.pop()
        assert popped is self._sem_poison
        nc.clear_and_free_semaphores(list(self.sems.allocated().values()))
        nc.all_engine_barrier()

    tile.TileContext._drain_and_barrier = _patched_drain_and_barrier
    tile.TileContext._drain_patch_installed = True


def _install_ntff_shim():
    """The image's antenv lacks axon_hooks, which silently degrades
    trace=True. Recreate the get/set pair and register the ctypes NTFF hook
    from trn_agent_boot (no-op if unavailable)."""
    if "antenv.axon_hooks" in sys.modules:
        return
    state = {"hook": None}

    def set_axon_ntff_profile_hook(h):
        state["hook"] = h

    def get_axon_ntff_profile_hook():
        return state["hook"]

    mod = types.ModuleType("antenv.axon_hooks")
    mod.set_axon_ntff_profile_hook = set_axon_ntff_profile_hook
    mod.get_axon_ntff_profile_hook = get_axon_ntff_profile_hook
    sys.modules["antenv.axon_hooks"] = mod
    try:
        import antenv

        antenv.axon_hooks = mod
        from trn_agent_boot.trn_boot import _ntff_profile_via_ctypes

        set_axon_ntff_profile_hook(
            _ntff_profile_via_ctypes("/opt/axon/libaxon_pjrt.so")
        )
    except Exception:
        pass


_install_tile_drain_patch()
_install_ntff_shim()

# ---------------------------------------------------------------------------
# Problem constants (hardcoded per the harness contract)
# ---------------------------------------------------------------------------

B, L, D = 4, 4096, 1024
N_CORES = 8
P = 128
LH = L // 2  # query rows per core
DC = D // P  # 8 contraction chunks of 128 over d/e
F32 = mybir.dt.float32
F32R = mybir.dt.float32r

ACHUNK = 512  # phase-A xT column chunk
BCHUNK = 512  # phase-B m-chunk (k/v rows)
PAIRS = [[2 * i, 2 * i + 1] for i in range(N_CORES // 2)]
BMS = BCHUNK // P
LBLOCK = 1024  # phase-B query-block rows


def build_nc():
    nc = bacc.Bacc("TRN2", target_bir_lowering=False, debug=False,
                   num_devices=N_CORES)
    xTr = nc.dram_tensor("xTr", [D, L], F32, kind="ExternalInput").ap()
    wqT = nc.dram_tensor("wqT", [D, D], F32, kind="ExternalInput").ap()
    wkT = nc.dram_tensor("wkT", [D, D], F32, kind="ExternalInput").ap()
    wvT = nc.dram_tensor("wvT", [D, D], F32, kind="ExternalInput").ap()
    out = nc.dram_tensor("out", [LH, D], F32, kind="ExternalOutput").ap()
    slots = nc.dram_tensor("slots", [1, 2], mybir.dt.uint32,
                           kind="ExternalInput").ap()
    kTsh = nc.dram_tensor("kTsh", [2, D, LH], F32R, addr_space="Shared").ap()
    vsh = nc.dram_tensor("vsh", [2, LH, D], F32R, addr_space="Shared").ap()
    tok = nc.dram_tensor("tok", [1, 2], F32).ap()
    tok2 = nc.dram_tensor("tok2", [1, 2], F32).ap()
    wu_sink = nc.dram_tensor("wu_sink", [P, ACHUNK], F32).ap()

    def chunked(ap):  # [K*, N] dram -> [P, K*/P, N] partition-major
        return ap.rearrange("(c p) n -> p c n", p=P)

    with tile.TileContext(nc) as tc, ExitStack() as octx:
        psum = octx.enter_context(tc.tile_pool(name="psum", bufs=8, space="PSUM"))
        qpool = octx.enter_context(tc.tile_pool(name="qpool", bufs=1))
        qsb = qpool.tile([P, DC, LH], F32R, tag="qsb")  # qT, SBUF-resident

        # HAM warmup: ~17us of junk matmuls while the first DMAs load, so the
        # PE clock gate is already at 8/8 when real work arrives
        with tc.tile_pool(name="wupool", bufs=1) as wupool:
            wut = wupool.tile([P, ACHUNK], F32R, tag="wut")
            nc.vector.memset(wut[:].bitcast(F32), 0.0)
            wuo = wupool.tile([P, ACHUNK], F32, tag="wuo")
            for g in range(20):
                wp = psum.tile([P, ACHUNK], F32, tag="ps", name=f"wu_{g}")
                for r in range(2):
                    nc.tensor.matmul(wp[:], wut[:, 0:P], wut[:],
                                     start=(r == 0), stop=(r == 1))
                if g == 19:
                    nc.vector.tensor_copy(wuo[:], wp[:])
            nc.sync.dma_start(wu_sink[:], wuo[:])

        # ---------------- Phase A: projections from one xT stream ----------
        # one interleaved loop (k, v, q per chunk — no intra-A transitions);
        # spills batched at 512KB so the sync ring's issue overhead fits the
        # per-chunk compute budget
        with ExitStack() as actx:
            wpool = actx.enter_context(tc.tile_pool(name="wpool", bufs=1))
            xpool = actx.enter_context(tc.tile_pool(name="xpool", bufs=2))
            stage = actx.enter_context(tc.tile_pool(name="stage", bufs=2))

            wq = wpool.tile([P, DC, D], F32R, tag="wq")
            wk = wpool.tile([P, DC, D], F32R, tag="wk")
            wv = wpool.tile([P, DC, D], F32R, tag="wv")
            # spread initial loads over rings: first MMs need wk + xc0 only,
            # and only their first c-chunks — split wk per chunk
            for c in range(DC):
                nc.sync.dma_start(wk[:, c], chunked(wkT).bitcast(F32R)[:, c])
            nc.gpsimd.dma_start(wv[:], chunked(wvT).bitcast(F32R))
            nc.gpsimd.dma_start(wq[:], chunked(wqT).bitcast(F32R))

            # rank-in-pair slot selectors for the shared spill buffers
            st_sl = stage.tile([1, 2], mybir.dt.uint32, tag="sl", bufs=1)
            nc.sync.dma_start(st_sl[:], slots[:])
            regs_o = nc.alloc_registers(
                engines=[EngineType.SP, EngineType.Activation])
            nc.regs_load(regs_o, st_sl[0:1, 0:1])
            svo = nc.snap(regs_o, donate=True)
            regs_p = nc.alloc_registers(
                engines=[EngineType.SP, EngineType.Activation])
            nc.regs_load(regs_p, st_sl[0:1, 1:2])
            svp = nc.snap(regs_p, donate=True)

            for j in range(LH // ACHUNK):
                xc = xpool.tile([P, DC, ACHUNK], F32R, tag="xc")
                cols = slice(j * ACHUNK, (j + 1) * ACHUNK)
                if j == 0:
                    # split per c-chunk: the first accumulation group starts
                    # after 768KB instead of 6MB of DMA
                    for c in range(DC):
                        nc.scalar.dma_start(
                            xc[:, c], chunked(xTr[:, cols]).bitcast(F32R)[:, c])
                else:
                    nc.scalar.dma_start(xc[:], chunked(xTr[:, cols]).bitcast(F32R))

                # kT chunk -> spill (four 512KB batches of 2 e-tiles)
                for eh in range(4):
                    kst = stage.tile([P, 2, ACHUNK], F32R, tag="st")
                    for ei in range(2):
                        e = eh * 2 + ei
                        pt = psum.tile([P, ACHUNK], F32, tag="ps")
                        for c in range(DC):
                            nc.tensor.matmul(
                                pt[:], wk[:, c, e * P:(e + 1) * P], xc[:, c],
                                start=(c == 0), stop=(c == DC - 1))
                        nc.vector.tensor_copy(kst[:, ei], pt[:])
                    nc.sync.dma_start(
                        kTsh[bass.ds(svo, 1), eh * 2 * P:(eh + 1) * 2 * P,
                             cols].rearrange("s (c p) n -> p (s c) n", p=P),
                        kst[:])

                # v chunk -> spill (four 512KB batches of 1 row-tile)
                for ms in range(ACHUNK // P):
                    row0 = j * ACHUNK + ms * P
                    vst = stage.tile([P, 1, D], F32R, tag="st", name=f"vst_{j}_{ms}")
                    for dh in range(D // ACHUNK):
                        pt = psum.tile([P, ACHUNK], F32, tag="ps")
                        dsl = slice(dh * ACHUNK, (dh + 1) * ACHUNK)
                        for c in range(DC):
                            nc.tensor.matmul(
                                pt[:], xc[:, c, ms * P:(ms + 1) * P],
                                wv[:, c, dsl],
                                start=(c == 0), stop=(c == DC - 1))
                        nc.vector.tensor_copy(vst[:, 0, dsl], pt[:])
                    nc.sync.dma_start(
                        vsh[bass.ds(svo, 1), row0:row0 + P, :].rearrange(
                            "s (c p) n -> p (s c) n", p=P),
                        vst[:])

                # qT chunk (first half of the rotated stream) -> resident
                if j < LH // ACHUNK:
                    for e in range(DC):
                        pt = psum.tile([P, ACHUNK], F32, tag="ps")
                        for c in range(DC):
                            nc.tensor.matmul(
                                pt[:], wq[:, c, e * P:(e + 1) * P], xc[:, c],
                                start=(c == 0), stop=(c == DC - 1))
                        nc.vector.tensor_copy(qsb[:, e, cols], pt[:])

            # pair barrier: the token is sampled from the shared buffers, so
            # its DMA carries a RAW dep on every spill write; the AllReduce
            # completes only when BOTH pair members' spills are durable
            tkt = stage.tile([1, 2], F32, tag="tkt", bufs=1)
            nc.sync.dma_start(tkt[0:1, 0:1], kTsh[bass.ds(svo, 1), 0:1, 0:1]
                              .rearrange("s c n -> c s n").bitcast(F32))
            nc.sync.dma_start(tkt[0:1, 1:2], vsh[bass.ds(svo, 1), 0:1, 0:1]
                              .rearrange("s c n -> c s n").bitcast(F32))
            nc.sync.dma_start(tok[:], tkt[:])
            pair_barrier = nc.gpsimd.collective_compute(
                "AllReduce", mybir.AluOpType.add, replica_groups=PAIRS,
                ins=[tok], outs=[tok2])

        # ---------------- Phase B: attention over m, single query block ----
        with ExitStack() as bctx:
            opool = bctx.enter_context(tc.tile_pool(name="opool", bufs=1))
            kpool = bctx.enter_context(tc.tile_pool(name="kpool", bufs=2))
            vpool = bctx.enter_context(tc.tile_pool(name="vpool", bufs=2))
            spool = bctx.enter_context(tc.tile_pool(name="spool", bufs=2))

            for lb in range(LH // LBLOCK):
                lsl0 = lb * LBLOCK
                ob = opool.tile([P, LBLOCK // P, D], F32, tag="ob")

                for j in range(L // BCHUNK):
                    # chunks 0-3: own half; 4-7: peer half (after the barrier)
                    own = j < LH // BCHUNK
                    sl = svo if own else svp
                    jj = j % (LH // BCHUNK)
                    msl = slice(jj * BCHUNK, (jj + 1) * BCHUNK)
                    kc = kpool.tile([P, DC, BCHUNK], F32R, tag="kc")
                    kld = nc.sync.dma_start(kc[:], kTsh[
                        bass.ds(sl, 1), :, msl].rearrange(
                        "s (c p) m -> p (s c) m", p=P))
                    vc = vpool.tile([P, BMS, D], F32R, tag="vc")
                    vld = nc.scalar.dma_start(vc[:], vsh[
                        bass.ds(sl, 1), msl, :].rearrange(
                        "s (c p) n -> p (s c) n", p=P))
                    if not own:
                        add_dep_helper(kld.ins, pair_barrier.ins,
                                       reason="peer kc after pair barrier")
                        add_dep_helper(vld.ins, pair_barrier.ins,
                                       reason="peer vc after pair barrier")

                    # sT chunk: [BCHUNK(m), LBLOCK(l)] as BMS tiles [P, LBLOCK]
                    sc = spool.tile([P, BMS, LBLOCK], F32R, tag="sc")
                    for ms in range(BMS):
                        for lh in range(LBLOCK // ACHUNK):
                            pt = psum.tile([P, ACHUNK], F32, tag="ps")
                            ls = slice(lh * ACHUNK, (lh + 1) * ACHUNK)
                            for e in range(DC):
                                nc.tensor.matmul(
                                    pt[:], kc[:, e, ms * P:(ms + 1) * P],
                                    qsb[:, e, lsl0 + lh * ACHUNK:
                                        lsl0 + (lh + 1) * ACHUNK],
                                    start=(e == 0), stop=(e == DC - 1))
                            nc.vector.tensor_copy(sc[:, ms, ls], pt[:])

                    # out += sT^T @ v, accumulated into ob
                    for lt in range(LBLOCK // P):
                        for dh in range(D // ACHUNK):
                            pt = psum.tile([P, ACHUNK], F32, tag="ps")
                            dsl = slice(dh * ACHUNK, (dh + 1) * ACHUNK)
                            for ms in range(BMS):
                                nc.tensor.matmul(
                                    pt[:], sc[:, ms, lt * P:(lt + 1) * P],
                                    vc[:, ms, dsl],
                                    start=(ms == 0), stop=(ms == BMS - 1))
                            if j == 0:
                                nc.vector.tensor_copy(ob[:, lt, dsl], pt[:])
                            else:
                                nc.vector.tensor_add(
                                    ob[:, lt, dsl], ob[:, lt, dsl], pt[:])

                # per-tile stores so the tail overlaps the last flush-adds
                for lt in range(LBLOCK // P):
                    row0 = lsl0 + lt * P
                    nc.sync.dma_start(out[row0:row0 + P, :], ob[:, lt])

    nc.compile()
    return nc


_NC_CACHE = {}


def _get_nc():
    if "nc" not in _NC_CACHE:
        _NC_CACHE["nc"] = build_nc()
    return _NC_CACHE["nc"]


def run(inputs, trace=False):
    """Run the kernel on all 8 cores. Returns (full_output, BassKernelResults)."""
    x = np.asarray(inputs["x"], dtype=np.float32)
    Wq = np.asarray(inputs["Wq"], dtype=np.float32)
    Wk = np.asarray(inputs["Wk"], dtype=np.float32)
    Wv = np.asarray(inputs["Wv"], dtype=np.float32)

    xT = np.ascontiguousarray(x.transpose(0, 2, 1))  # [B, D, L]
    inv_sqrt_d = np.float32(1.0 / np.sqrt(D))
    wqT = np.ascontiguousarray(Wq.T * inv_sqrt_d)
    wkT = np.ascontiguousarray(Wk.T)
    wvT = np.ascontiguousarray(Wv.T)

    in_maps = []
    for c in range(N_CORES):
        b, h = c // 2, c % 2
        # rotate columns so this core's own half comes first
        xtb = xT[b]
        xtr = np.concatenate(
            [xtb[:, h * LH:(h + 1) * LH], xtb[:, (1 - h) * LH:(2 - h) * LH]],
            axis=1)
        in_maps.append({
            "xTr": np.ascontiguousarray(xtr),
            "slots": np.array([[h, 1 - h]], dtype=np.uint32),
            "wqT": wqT, "wkT": wkT, "wvT": wvT,
        })

    nc = _get_nc()
    res = run_bass_kernel_spmd(nc, in_maps, list(range(N_CORES)), trace=trace)

    full = np.empty((B, L, D), dtype=np.float32)
    for c in range(N_CORES):
        b, h = c // 2, c % 2
        full[b, h * LH:(h + 1) * LH, :] = res.results[c]["out"]
    return full, res


def kernel(**inputs):
    full, _ = run(inputs, trace=False)
    return full



# revision 9
# speedup vs baseline: 2.7374x; 2.7374x over previous
"""Trainium2 Bass kernel for softmax-free attention:
    q = x @ Wq^T; k = x @ Wk^T; v = x @ Wv^T
    s = (q @ k^T) / sqrt(d); out = s @ v
  x: [4, 4096, 1024], W*: [1024, 1024], out: [4, 4096, 1024] (fp32)

KEY RESTRUCTURE: there is no softmax, so the chain is associative:
    out = q (k^T v) / sqrt(d) = x @ [Wq^T Wk (x^T x) Wv^T] / sqrt(d)
With G_b = x_b^T x_b (per batch), the per-core FLOPs drop from ~47 GF
(score-matrix path, PE-bound ~600us) to ~15 GF (~190us):
    AT = Wk^T Wq-style product (A^T where A = Wq^T Wk)   [input-only]
    G  = x_own^T x_own  (+ pair exchange over the 2 cores of a batch)
    M  = G @ WvT'        (WvT' = Wv^T / sqrt(d), folded on host)
    N  = A @ M
    out = x_own @ N
All on-chip operands are bf16 (same 78.6 TF/s PE rate as fp32r, ~5e-3
end-to-end rel err vs the 2e-2 gate; halves SBUF + DMA), accumulation in
fp32 PSUM with 8-16 deep groups into [128,512] tiles.

Sharding: core c handles batch c//2, sequence-half c%2 (2048 rows). The
only cross-core data is the 2MB partial-G exchange within each pair,
via cross-core-visible Shared-DRAM spills (slot = rank-in-pair via a
dynamic DMA offset) ordered by a tiny token AllReduce. The exchange is
hidden under the AT matmuls; the peer partial is summed into G on
GPSIMD so the DVE copy stream never blocks.

Layouts (PE contracts over the partition dim):
    G[e,f]   = sum_l  xn[l,e] xn[l,f]      lhsT=xn chunk, rhs=xn chunk
    AT[c,a]  = sum_b  Wk[b,c] Wq[b,a]      lhsT=Wk chunk, rhs=Wq chunk
    M[c,d]   = sum_f  G[f,c]  WvT'[f,d]    lhsT=G chunk (G symmetric)
    N[a,d]   = sum_c  AT[c,a] M[c,d]       lhsT=AT chunk
    out[l,d] = sum_a  xT[a,l] N[a,d]       lhsT=xT chunk (host transpose)
xn and xT share one rotating SBUF buffer (xT is only needed after the
last xn read; the WAR dep is automatic).
"""

import sys
import types
from contextlib import ExitStack

import numpy as np
from ml_dtypes import bfloat16

import concourse.bass as bass
import concourse.tile as tile
from concourse import bacc, mybir
from concourse.bass_utils import run_bass_kernel_spmd
from concourse.mybir import EngineType
from concourse.tile import add_dep_helper
from concourse.vector_clock import ScopedClock

# ---------------------------------------------------------------------------
# Environment shims
# ---------------------------------------------------------------------------


def _install_tile_drain_patch():
    """This toolchain's walrus caps sync waits at 1 per instruction, but
    TileContext's tail drain can carry several. Split the overflow onto
    preceding nops (same semantics: the issuing engine observes every sem
    before draining)."""
    if getattr(tile.TileContext, "_drain_patch_installed", False):
        return

    def _patched_drain_and_barrier(self, tick_clock, wait_clock):
        nc = self.nc
        collector = nc.sync.nop(hint="drain_wait_collector", nofuse=True)
        wait_clock.add_sem_waits(
            collector.ins, ScopedClock({None: tick_clock.global_clock})
        )
        waits = list(collector.ins.sync_info.on_wait or [])
        if len(waits) > 1:
            collector.ins.sync_info.on_wait = [waits[0]]
            for w in waits[1:]:
                nop = nc.sync.nop(hint="drain_wait_extra", nofuse=True)
                nop.ins.sync_info = mybir.SyncInfo(on_wait=[w], on_update=[])
        nc.sync.drain()

        nc.all_engine_barrier()
        assert self.sems is not None
        popped = nc._tile_sem_poison_stack.pop()
        assert popped is self._sem_poison
        nc.clear_and_free_semaphores(list(self.sems.allocated().values()))
        nc.all_engine_barrier()

    tile.TileContext._drain_and_barrier = _patched_drain_and_barrier
    tile.TileContext._drain_patch_installed = True


def _install_ntff_shim():
    """The image's antenv lacks axon_hooks, which silently degrades
    trace=True. Recreate the get/set pair and register the ctypes NTFF hook
    from trn_agent_boot (no-op if unavailable)."""
    if "antenv.axon_hooks" in sys.modules:
        return
    state = {"hook": None}

    def set_axon_ntff_profile_hook(h):
        state["hook"] = h

    def get_axon_ntff_profile_hook():
        return state["hook"]

    mod = types.ModuleType("antenv.axon_hooks")
    mod.set_axon_ntff_profile_hook = set_axon_ntff_profile_hook
    mod.get_axon_ntff_profile_hook = get_axon_ntff_profile_hook
    sys.modules["antenv.axon_hooks"] = mod
    try:
        import antenv

        antenv.axon_hooks = mod
        from trn_agent_boot.trn_boot import _ntff_profile_via_ctypes

        set_axon_ntff_profile_hook(
            _ntff_profile_via_ctypes("/opt/axon/libaxon_pjrt.so")
        )
    except Exception:
        pass


_install_tile_drain_patch()
_install_ntff_shim()

# ---------------------------------------------------------------------------
# Problem constants (hardcoded per the harness contract)
# ---------------------------------------------------------------------------

B, L, D = 4, 4096, 1024
N_CORES = 8
P = 128
LH = L // 2          # rows per core
DC = D // P          # 8 chunks of 128 over d/e/c/a/f
LC = LH // P         # 16 l-chunks of the natural-layout x
FREE = 512           # PSUM tile free dim (one bank, fp32 accum)
F32 = mybir.dt.float32
BF16 = mybir.dt.bfloat16
PAIRS = [[2 * i, 2 * i + 1] for i in range(N_CORES // 2)]
WU_GROUPS = 26       # HAM warmup groups (~22us at cold clock)


def build_nc():
    nc = bacc.Bacc("TRN2", target_bir_lowering=False, debug=False,
                   num_devices=N_CORES)
    xn = nc.dram_tensor("xn", [LH, D], BF16, kind="ExternalInput").ap()
    xT = nc.dram_tensor("xT", [D, LH], BF16, kind="ExternalInput").ap()
    wq = nc.dram_tensor("wq", [D, D], BF16, kind="ExternalInput").ap()
    wk = nc.dram_tensor("wk", [D, D], BF16, kind="ExternalInput").ap()
    wvT = nc.dram_tensor("wvT", [D, D], BF16, kind="ExternalInput").ap()
    out = nc.dram_tensor("out", [LH, D], F32, kind="ExternalOutput").ap()
    slots = nc.dram_tensor("slots", [1, 2], mybir.dt.uint32,
                           kind="ExternalInput").ap()
    Gsh = nc.dram_tensor("Gsh", [2, D, D], BF16, addr_space="Shared").ap()
    tok = nc.dram_tensor("tok", [1, 2], BF16).ap()
    tok2 = nc.dram_tensor("tok2", [1, 2], BF16).ap()
    wu_sink = nc.dram_tensor("wu_sink", [P, FREE], F32).ap()

    def chunked(ap):  # [K*, N] dram -> [P, K*/P, N] partition-major
        return ap.rearrange("(c p) n -> p c n", p=P)

    with tile.TileContext(nc) as tc, ExitStack() as octx:
        psum = octx.enter_context(tc.tile_pool(name="psum", bufs=8, space="PSUM"))
        tokp = octx.enter_context(tc.tile_pool(name="tokp", bufs=1))

        # HAM warmup: junk matmuls while the first DMAs load, so the PE
        # clock gate is already at 8/8 when real work arrives
        wut = tokp.tile([P, FREE], BF16, tag="wut")
        nc.vector.memset(wut[:].bitcast(mybir.dt.uint16), 0)
        wuo = tokp.tile([P, FREE], F32, tag="wuo")
        for g in range(WU_GROUPS):
            wp = psum.tile([P, FREE], F32, tag="ps", name=f"wu_{g}")
            for r in range(2):
                nc.tensor.matmul(wp[:], wut[:, 0:P], wut[:],
                                 start=(r == 0), stop=(r == 1))
            if g == WU_GROUPS - 1:
                nc.vector.tensor_copy(wuo[:], wp[:])
        nc.sync.dma_start(wu_sink[:], wuo[:])

        # ---------------- persistent SBUF tiles + loads --------------------
        xbig_pool = octx.enter_context(tc.tile_pool(name="xbig", bufs=1))
        xnt = xbig_pool.tile([P, LC, D], BF16, tag="xb")  # xn natural
        for h in range(4):  # sync ring, 4 x 1MB
            nc.sync.dma_start(xnt[:, 4 * h:4 * (h + 1)],
                              chunked(xn)[:, 4 * h:4 * (h + 1)])
        wpool = octx.enter_context(tc.tile_pool(name="wpool", bufs=1))
        wqt = wpool.tile([P, DC, D], BF16, tag="wq")
        wkt = wpool.tile([P, DC, D], BF16, tag="wk")
        wvt = wpool.tile([P, DC, D], BF16, tag="wv")
        nc.gpsimd.dma_start(wkt[:], chunked(wk))
        nc.gpsimd.dma_start(wqt[:], chunked(wq))
        nc.scalar.dma_start(wvt[:], chunked(wvT))
        gpool = octx.enter_context(tc.tile_pool(name="gpool", bufs=1))
        gsb = gpool.tile([P, DC, D], BF16, tag="g")
        atsb = gpool.tile([P, DC, D], BF16, tag="at")
        msb = gpool.tile([P, DC, D], BF16, tag="m")
        nsb = gpool.tile([P, DC, D], BF16, tag="n")
        gpp = octx.enter_context(tc.tile_pool(name="gpp", bufs=2))
        opool = octx.enter_context(tc.tile_pool(name="opool", bufs=4))

        st_sl = tokp.tile([1, 2], mybir.dt.uint32, tag="sl", bufs=1)
        nc.sync.dma_start(st_sl[:], slots[:])
        regs_o = nc.alloc_registers(
            engines=[EngineType.SP, EngineType.Activation])
        nc.regs_load(regs_o, st_sl[0:1, 0:1])
        svo = nc.snap(regs_o, donate=True)
        regs_p = nc.alloc_registers(
            engines=[EngineType.SP, EngineType.Activation])
        nc.regs_load(regs_p, st_sl[0:1, 1:2])
        svp = nc.snap(regs_p, donate=True)

        # ---------------- G = xn^T xn, spilled per e-chunk -----------------
        for ec in range(DC):
            for fh in range(2):
                fsl = slice(fh * FREE, (fh + 1) * FREE)
                pt = psum.tile([P, FREE], F32, tag="ps")
                for lc in range(LC):
                    nc.tensor.matmul(
                        pt[:], xnt[:, lc, ec * P:(ec + 1) * P],
                        xnt[:, lc, fh * FREE:(fh + 1) * FREE],
                        start=(lc == 0), stop=(lc == LC - 1))
                nc.vector.tensor_copy(gsb[:, ec, fsl], pt[:])
            nc.sync.dma_start(
                Gsh[bass.ds(svo, 1), ec * P:(ec + 1) * P, :].rearrange(
                    "s (c p) n -> p (s c) n", p=P),
                gsb[:, ec:ec + 1])

        # pair barrier: token sampled from the shared buffer carries a RAW
        # dep on the spill writes; the AllReduce completes only when BOTH
        # pair members' spills are durable
        tkt = tokp.tile([1, 2], BF16, tag="tkt", bufs=1)
        nc.sync.dma_start(tkt[0:1, 0:1], Gsh[bass.ds(svo, 1), 0:1, 0:1]
                          .rearrange("s c n -> c s n"))
        nc.sync.dma_start(tkt[0:1, 1:2],
                          Gsh[bass.ds(svo, 1), D - 1:D, 0:1]
                          .rearrange("s c n -> c s n"))
        nc.sync.dma_start(tok[:], tkt[:])
        pair_barrier = nc.gpsimd.collective_compute(
            "AllReduce", mybir.AluOpType.add, replica_groups=PAIRS,
            ins=[tok], outs=[tok2])

        # peer partial-G: read per chunk (scalar ring), sum on GPSIMD so the
        # DVE copy stream (G/AT tiles) is never blocked behind these
        for ec in range(DC):
            gp = gpp.tile([P, 1, D], BF16, tag="gp")
            rd = nc.scalar.dma_start(
                gp[:], Gsh[bass.ds(svp, 1), ec * P:(ec + 1) * P, :].rearrange(
                    "s (c p) n -> p (s c) n", p=P))
            add_dep_helper(rd.ins, pair_barrier.ins,
                           reason="peer G after pair barrier")
            nc.gpsimd.tensor_add(gsb[:, ec], gsb[:, ec], gp[:, 0])

        # ---------------- AT[c,a] = sum_b Wk[b,c] Wq[b,a] ------------------
        for cc in range(DC):
            for ah in range(2):
                asl = slice(ah * FREE, (ah + 1) * FREE)
                pt = psum.tile([P, FREE], F32, tag="ps")
                for bb in range(DC):
                    nc.tensor.matmul(
                        pt[:], wkt[:, bb, cc * P:(cc + 1) * P],
                        wqt[:, bb, asl],
                        start=(bb == 0), stop=(bb == DC - 1))
                nc.vector.tensor_copy(atsb[:, cc, asl], pt[:])

        # xT reuses xn's buffer (WAR: waits for the last G matmul read)
        xtt = xbig_pool.tile([P, DC, LH], BF16, tag="xb")  # xT chunked
        for h in range(2):  # scalar ring: after peer-G reads; needed at out
            nc.scalar.dma_start(xtt[:, 4 * h:4 * (h + 1)],
                                chunked(xT)[:, 4 * h:4 * (h + 1)])

        # ---------------- M[c,d] = sum_f G[f,c] WvT'[f,d] ------------------
        for cc in range(DC):
            for dh in range(2):
                dsl = slice(dh * FREE, (dh + 1) * FREE)
                pt = psum.tile([P, FREE], F32, tag="ps")
                for fc in range(DC):
                    nc.tensor.matmul(
                        pt[:], gsb[:, fc, cc * P:(cc + 1) * P],
                        wvt[:, fc, dsl],
                        start=(fc == 0), stop=(fc == DC - 1))
                nc.vector.tensor_copy(msb[:, cc, dsl], pt[:])

        # ---------------- N[a,d] = sum_c AT[c,a] M[c,d] --------------------
        for ac in range(DC):
            for dh in range(2):
                dsl = slice(dh * FREE, (dh + 1) * FREE)
                pt = psum.tile([P, FREE], F32, tag="ps")
                for cc in range(DC):
                    nc.tensor.matmul(
                        pt[:], atsb[:, cc, ac * P:(ac + 1) * P],
                        msb[:, cc, dsl],
                        start=(cc == 0), stop=(cc == DC - 1))
                nc.vector.tensor_copy(nsb[:, ac, dsl], pt[:])

        # ---------------- out[l,d] = sum_a xT[a,l] N[a,d] ------------------
        for lt in range(LC):
            ob = opool.tile([P, D], F32, tag="ob")
            for dh in range(2):
                dsl = slice(dh * FREE, (dh + 1) * FREE)
                pt = psum.tile([P, FREE], F32, tag="ps")
                for ac in range(DC):
                    nc.tensor.matmul(
                        pt[:], xtt[:, ac, lt * P:(lt + 1) * P],
                        nsb[:, ac, dsl],
                        start=(ac == 0), stop=(ac == DC - 1))
                nc.vector.tensor_copy(ob[:, dsl], pt[:])
            nc.sync.dma_start(out[lt * P:(lt + 1) * P, :], ob[:])

    nc.compile()
    return nc


_NC_CACHE = {}


def _get_nc():
    if "nc" not in _NC_CACHE:
        _NC_CACHE["nc"] = build_nc()
    return _NC_CACHE["nc"]


def run(inputs, trace=False):
    """Run the kernel on all 8 cores. Returns (full_output, BassKernelResults)."""
    x = np.asarray(inputs["x"], dtype=np.float32)
    Wq = np.asarray(inputs["Wq"], dtype=np.float32)
    Wk = np.asarray(inputs["Wk"], dtype=np.float32)
    Wv = np.asarray(inputs["Wv"], dtype=np.float32)

    inv_sqrt_d = np.float32(1.0 / np.sqrt(D))
    wq_h = np.ascontiguousarray(Wq.astype(bfloat16))
    wk_h = np.ascontiguousarray(Wk.astype(bfloat16))
    wvT_h = np.ascontiguousarray((Wv.T * inv_sqrt_d).astype(bfloat16))
    xb = x.astype(bfloat16)

    in_maps = []
    for c in range(N_CORES):
        b, h = c // 2, c % 2
        rows = slice(h * LH, (h + 1) * LH)
        in_maps.append({
            "xn": np.ascontiguousarray(xb[b, rows, :]),
            "xT": np.ascontiguousarray(xb[b].T[:, rows]),
            "slots": np.array([[h, 1 - h]], dtype=np.uint32),
            "wq": wq_h, "wk": wk_h, "wvT": wvT_h,
        })

    nc = _get_nc()
    res = run_bass_kernel_spmd(nc, in_maps, list(range(N_CORES)), trace=trace)

    full = np.empty((B, L, D), dtype=np.float32)
    for c in range(N_CORES):
        b, h = c // 2, c % 2
        full[b, h * LH:(h + 1) * LH, :] = res.results[c]["out"]
    return full, res


def kernel(**inputs):
    full, _ = run(inputs, trace=False)
    return full


# revision 10
# speedup vs baseline: 3.3982x; 1.2414x over previous
"""Trainium2 Bass kernel for softmax-free attention:
    q = x @ Wq^T; k = x @ Wk^T; v = x @ Wv^T
    s = (q @ k^T) / sqrt(d); out = s @ v
  x: [4, 4096, 1024], W*: [1024, 1024], out: [4, 4096, 1024] (fp32)

KEY RESTRUCTURE: there is no softmax, so the chain is associative:
    out = q (k^T v) / sqrt(d) = x @ [Wq^T Wk (x^T x) Wv^T] / sqrt(d)
With G_b = x_b^T x_b (per batch), the per-core FLOPs drop from ~47 GF
(score-matrix path, PE-bound ~600us) to ~12 GF:
    G  = x_own^T x_own  (+ pair exchange over the 2 cores of a batch)
    AT = A^T where A = Wq^T Wk                             [input-only]
    M  = G @ WvT'[:, own d-half]   (WvT' = Wv^T/sqrt(d), host-folded)
    N  = A @ M                      (column split propagates for free)
    out = x_own @ [N_own | N_peer]
M and N are computed only for this core's 512 d-columns; the 1MB
N-half exchange rides the same Shared-DRAM + token-AllReduce machinery
as the G exchange and hides under the out phase's own-half matmuls.
WvT' columns are rotated per-core on the host so "own half" is always
columns 0:512 (SPMD program stays identical across cores); the host
un-rotates the output columns after gather.

All on-chip operands are bf16 (same 78.6 TF/s PE rate as fp32r, ~5e-3
end-to-end rel err vs the 2e-2 gate), fp32 PSUM accumulation, 8-16
deep groups into [128,512] tiles. All input loads are serialized on
the sync ring behind xn so G (the first PE phase) is never starved;
the HAM warmup covers the xn load latency.

Exchange ordering: a token DMA samples the Shared spill region (RAW
dep on all spill writes, dynamic-slot aliasing) straight into the
collective's DRAM input; the tiny pair AllReduce completes only when
both members' spills are durable. Peer G partials are summed into G
on GPSIMD during the AT phase so the DVE copy stream never blocks.
"""

import sys
import types
from contextlib import ExitStack

import numpy as np
from ml_dtypes import bfloat16

import concourse.bass as bass
import concourse.tile as tile
from concourse import bacc, mybir
from concourse.bass_utils import run_bass_kernel_spmd
from concourse.mybir import EngineType
from concourse.tile import add_dep_helper
from concourse.vector_clock import ScopedClock

# ---------------------------------------------------------------------------
# Environment shims
# ---------------------------------------------------------------------------


def _install_tile_drain_patch():
    """This toolchain's walrus caps sync waits at 1 per instruction, but
    TileContext's tail drain can carry several. Split the overflow onto
    preceding nops (same semantics: the issuing engine observes every sem
    before draining)."""
    if getattr(tile.TileContext, "_drain_patch_installed", False):
        return

    def _patched_drain_and_barrier(self, tick_clock, wait_clock):
        nc = self.nc
        collector = nc.sync.nop(hint="drain_wait_collector", nofuse=True)
        wait_clock.add_sem_waits(
            collector.ins, ScopedClock({None: tick_clock.global_clock})
        )
        waits = list(collector.ins.sync_info.on_wait or [])
        if len(waits) > 1:
            collector.ins.sync_info.on_wait = [waits[0]]
            for w in waits[1:]:
                nop = nc.sync.nop(hint="drain_wait_extra", nofuse=True)
                nop.ins.sync_info = mybir.SyncInfo(on_wait=[w], on_update=[])
        nc.sync.drain()

        nc.all_engine_barrier()
        assert self.sems is not None
        popped = nc._tile_sem_poison_stack.pop()
        assert popped is self._sem_poison
        nc.clear_and_free_semaphores(list(self.sems.allocated().values()))
        nc.all_engine_barrier()

    tile.TileContext._drain_and_barrier = _patched_drain_and_barrier
    tile.TileContext._drain_patch_installed = True


def _install_ntff_shim():
    """The image's antenv lacks axon_hooks, which silently degrades
    trace=True. Recreate the get/set pair and register the ctypes NTFF hook
    from trn_agent_boot (no-op if unavailable)."""
    if "antenv.axon_hooks" in sys.modules:
        return
    state = {"hook": None}

    def set_axon_ntff_profile_hook(h):
        state["hook"] = h

    def get_axon_ntff_profile_hook():
        return state["hook"]

    mod = types.ModuleType("antenv.axon_hooks")
    mod.set_axon_ntff_profile_hook = set_axon_ntff_profile_hook
    mod.get_axon_ntff_profile_hook = get_axon_ntff_profile_hook
    sys.modules["antenv.axon_hooks"] = mod
    try:
        import antenv

        antenv.axon_hooks = mod
        from trn_agent_boot.trn_boot import _ntff_profile_via_ctypes

        set_axon_ntff_profile_hook(
            _ntff_profile_via_ctypes("/opt/axon/libaxon_pjrt.so")
        )
    except Exception:
        pass


_install_tile_drain_patch()
_install_ntff_shim()

# ---------------------------------------------------------------------------
# Problem constants (hardcoded per the harness contract)
# ---------------------------------------------------------------------------

B, L, D = 4, 4096, 1024
N_CORES = 8
P = 128
LH = L // 2          # rows per core
DC = D // P          # 8 chunks of 128 over d/e/c/a/f
LC = LH // P         # 16 l-chunks of the natural-layout x
FREE = 512           # PSUM tile free dim (one bank, fp32 accum)
F32 = mybir.dt.float32
BF16 = mybir.dt.bfloat16
PAIRS = [[2 * i, 2 * i + 1] for i in range(N_CORES // 2)]
WU_GROUPS = 14       # HAM warmup groups (~12us at cold clock)


def build_nc():
    nc = bacc.Bacc("TRN2", target_bir_lowering=False, debug=False,
                   num_devices=N_CORES)
    xn = nc.dram_tensor("xn", [LH, D], BF16, kind="ExternalInput").ap()
    xT = nc.dram_tensor("xT", [D, LH], BF16, kind="ExternalInput").ap()
    wq = nc.dram_tensor("wq", [D, D], BF16, kind="ExternalInput").ap()
    wk = nc.dram_tensor("wk", [D, D], BF16, kind="ExternalInput").ap()
    wvT = nc.dram_tensor("wvT", [D, FREE], BF16, kind="ExternalInput").ap()
    out = nc.dram_tensor("out", [LH, D], F32, kind="ExternalOutput").ap()
    slots = nc.dram_tensor("slots", [1, 2], mybir.dt.uint32,
                           kind="ExternalInput").ap()
    Gsh = nc.dram_tensor("Gsh", [2, D, D], BF16, addr_space="Shared").ap()
    Nsh = nc.dram_tensor("Nsh", [2, D, FREE], BF16, addr_space="Shared").ap()
    tok = nc.dram_tensor("tok", [1, 2], BF16).ap()
    tok2 = nc.dram_tensor("tok2", [1, 2], BF16).ap()
    tok3 = nc.dram_tensor("tok3", [1, 2], BF16).ap()
    tok4 = nc.dram_tensor("tok4", [1, 2], BF16).ap()
    wu_sink = nc.dram_tensor("wu_sink", [P, FREE], F32).ap()

    def chunked(ap):  # [K*, N] dram -> [P, K*/P, N] partition-major
        return ap.rearrange("(c p) n -> p c n", p=P)

    with tile.TileContext(nc) as tc, ExitStack() as octx:
        psum = octx.enter_context(tc.tile_pool(name="psum", bufs=8, space="PSUM"))
        tokp = octx.enter_context(tc.tile_pool(name="tokp", bufs=1))

        # HAM warmup: junk matmuls while xn loads, so the PE clock gate is
        # already at 8/8 when real work arrives
        wut = tokp.tile([P, FREE], BF16, tag="wut")
        nc.vector.memset(wut[:].bitcast(mybir.dt.uint16), 0)
        wuo = tokp.tile([P, FREE], F32, tag="wuo")
        for g in range(WU_GROUPS):
            wp = psum.tile([P, FREE], F32, tag="ps", name=f"wu_{g}")
            for r in range(2):
                nc.tensor.matmul(wp[:], wut[:, 0:P], wut[:],
                                 start=(r == 0), stop=(r == 1))
            if g == WU_GROUPS - 1:
                nc.vector.tensor_copy(wuo[:], wp[:])
        nc.gpsimd.dma_start(wu_sink[:], wuo[:])

        # slot registers (first on the sync ring: 8 bytes)
        st_sl = tokp.tile([1, 2], mybir.dt.uint32, tag="sl", bufs=1)
        nc.sync.dma_start(st_sl[:], slots[:])
        regs_o = nc.alloc_registers(
            engines=[EngineType.SP, EngineType.Activation])
        nc.regs_load(regs_o, st_sl[0:1, 0:1])
        svo = nc.snap(regs_o, donate=True)
        regs_p = nc.alloc_registers(
            engines=[EngineType.SP, EngineType.Activation])
        nc.regs_load(regs_p, st_sl[0:1, 1:2])
        svp = nc.snap(regs_p, donate=True)

        # persistent SBUF tiles; ALL input loads serialized on the sync ring
        # in need-order (xn feeds the first PE phase)
        xbig_pool = octx.enter_context(tc.tile_pool(name="xbig", bufs=1))
        xnt = xbig_pool.tile([P, LC, D], BF16, tag="xb")  # xn natural
        for h in range(4):
            nc.sync.dma_start(xnt[:, 4 * h:4 * (h + 1)],
                              chunked(xn)[:, 4 * h:4 * (h + 1)])
        wpool = octx.enter_context(tc.tile_pool(name="wpool", bufs=1))
        wqt = wpool.tile([P, DC, D], BF16, tag="wq")
        wkt = wpool.tile([P, DC, D], BF16, tag="wk")
        wvt = wpool.tile([P, DC, FREE], BF16, tag="wv")
        nc.sync.dma_start(wkt[:], chunked(wk))
        nc.sync.dma_start(wqt[:], chunked(wq))
        nc.sync.dma_start(wvt[:], chunked(wvT))
        gpool = octx.enter_context(tc.tile_pool(name="gpool", bufs=1))
        gsb = gpool.tile([P, DC, D], BF16, tag="g")
        atsb = gpool.tile([P, DC, D], BF16, tag="at")
        msb = gpool.tile([P, DC, FREE], BF16, tag="m")
        nsb = gpool.tile([P, DC, D], BF16, tag="n")
        gpp = octx.enter_context(tc.tile_pool(name="gpp", bufs=2))
        opool = octx.enter_context(tc.tile_pool(name="opool", bufs=4))

        # ---------------- G = xn^T xn, spilled per e-chunk -----------------
        for ec in range(DC):
            for fh in range(2):
                fsl = slice(fh * FREE, (fh + 1) * FREE)
                pt = psum.tile([P, FREE], F32, tag="ps")
                for lc in range(LC):
                    nc.tensor.matmul(
                        pt[:], xnt[:, lc, ec * P:(ec + 1) * P],
                        xnt[:, lc, fsl],
                        start=(lc == 0), stop=(lc == LC - 1))
                nc.vector.tensor_copy(gsb[:, ec, fsl], pt[:])
            nc.sync.dma_start(
                Gsh[bass.ds(svo, 1), ec * P:(ec + 1) * P, :].rearrange(
                    "s (c p) n -> p (s c) n", p=P),
                gsb[:, ec:ec + 1])

        # pair barrier #1: DRAM->DRAM token sample (RAW dep on the spills
        # via the dynamic-slot alias), then a tiny AllReduce
        nc.sync.dma_start(
            tok[:], Gsh[bass.ds(svo, 1), 0:1, 0:2]
            .rearrange("s c n -> c (s n)"))
        pair_barrier = nc.gpsimd.collective_compute(
            "AllReduce", mybir.AluOpType.add, replica_groups=PAIRS,
            ins=[tok], outs=[tok2])

        # peer partial-G: read per chunk (scalar ring), sum on GPSIMD so the
        # DVE copy stream (G/AT tiles) is never blocked behind these
        for ec in range(DC):
            gp = gpp.tile([P, 1, D], BF16, tag="gp")
            rd = nc.scalar.dma_start(
                gp[:], Gsh[bass.ds(svp, 1), ec * P:(ec + 1) * P, :].rearrange(
                    "s (c p) n -> p (s c) n", p=P))
            add_dep_helper(rd.ins, pair_barrier.ins,
                           reason="peer G after pair barrier")
            nc.gpsimd.tensor_add(gsb[:, ec], gsb[:, ec], gp[:, 0])

        # ---------------- AT[c,a] = sum_b Wk[b,c] Wq[b,a] ------------------
        # (runs while the G exchange completes; M waits on the adds)
        for cc in range(DC):
            for ah in range(2):
                asl = slice(ah * FREE, (ah + 1) * FREE)
                pt = psum.tile([P, FREE], F32, tag="ps")
                for bb in range(DC):
                    nc.tensor.matmul(
                        pt[:], wkt[:, bb, cc * P:(cc + 1) * P],
                        wqt[:, bb, asl],
                        start=(bb == 0), stop=(bb == DC - 1))
                nc.vector.tensor_copy(atsb[:, cc, asl], pt[:])

        # xT reuses xn's buffer (WAR: waits for the last G matmul read)
        xtt = xbig_pool.tile([P, DC, LH], BF16, tag="xb")  # xT chunked
        for h in range(2):  # scalar ring: after peer-G reads; needed at out
            nc.scalar.dma_start(xtt[:, 4 * h:4 * (h + 1)],
                                chunked(xT)[:, 4 * h:4 * (h + 1)])

        # ------------- M[c, own d-half] = sum_f G[f,c] WvT'[f, d] ----------
        for cc in range(DC):
            pt = psum.tile([P, FREE], F32, tag="ps")
            for fc in range(DC):
                nc.tensor.matmul(
                    pt[:], gsb[:, fc, cc * P:(cc + 1) * P],
                    wvt[:, fc],
                    start=(fc == 0), stop=(fc == DC - 1))
            nc.vector.tensor_copy(msb[:, cc], pt[:])

        # ------------- N[a, own d-half] = sum_c AT[c,a] M[c,d] -------------
        for ac in range(DC):
            pt = psum.tile([P, FREE], F32, tag="ps")
            for cc in range(DC):
                nc.tensor.matmul(
                    pt[:], atsb[:, cc, ac * P:(ac + 1) * P],
                    msb[:, cc],
                    start=(cc == 0), stop=(cc == DC - 1))
            nc.vector.tensor_copy(nsb[:, ac, 0:FREE], pt[:])

        # N-half exchange: spill own half, barrier #2, read peer half into
        # the high columns of nsb (hidden under the out own-half matmuls)
        nc.sync.dma_start(
            Nsh[bass.ds(svo, 1), :, :].rearrange("s (c p) n -> p (s c) n",
                                                 p=P),
            nsb[:, :, 0:FREE])
        nc.sync.dma_start(
            tok3[:], Nsh[bass.ds(svo, 1), 0:1, 0:2]
            .rearrange("s c n -> c (s n)"))
        pair_barrier2 = nc.gpsimd.collective_compute(
            "AllReduce", mybir.AluOpType.add, replica_groups=PAIRS,
            ins=[tok3], outs=[tok4])
        nrd = nc.scalar.dma_start(
            nsb[:, :, FREE:D],
            Nsh[bass.ds(svp, 1), :, :].rearrange("s (c p) n -> p (s c) n",
                                                 p=P))
        add_dep_helper(nrd.ins, pair_barrier2.ins,
                       reason="peer N after pair barrier 2")

        # ---------------- out[l,d] = sum_a xT[a,l] N[a,d] ------------------
        # own d-half first (no peer dep), peer half second
        for dh in range(2):
            dsl = slice(dh * FREE, (dh + 1) * FREE)
            for lt in range(LC):
                ob = opool.tile([P, FREE], F32, tag="ob")
                pt = psum.tile([P, FREE], F32, tag="ps")
                for ac in range(DC):
                    nc.tensor.matmul(
                        pt[:], xtt[:, ac, lt * P:(lt + 1) * P],
                        nsb[:, ac, dsl],
                        start=(ac == 0), stop=(ac == DC - 1))
                nc.vector.tensor_copy(ob[:], pt[:])
                nc.sync.dma_start(out[lt * P:(lt + 1) * P, dsl], ob[:])

    nc.compile()
    return nc


_NC_CACHE = {}


def _get_nc():
    if "nc" not in _NC_CACHE:
        _NC_CACHE["nc"] = build_nc()
    return _NC_CACHE["nc"]


def run(inputs, trace=False):
    """Run the kernel on all 8 cores. Returns (full_output, BassKernelResults)."""
    x = np.asarray(inputs["x"], dtype=np.float32)
    Wq = np.asarray(inputs["Wq"], dtype=np.float32)
    Wk = np.asarray(inputs["Wk"], dtype=np.float32)
    Wv = np.asarray(inputs["Wv"], dtype=np.float32)

    inv_sqrt_d = np.float32(1.0 / np.sqrt(D))
    wq_h = np.ascontiguousarray(Wq.astype(bfloat16))
    wk_h = np.ascontiguousarray(Wk.astype(bfloat16))
    wvT_f = (Wv.T * inv_sqrt_d).astype(bfloat16)
    xb = x.astype(bfloat16)

    in_maps = []
    for c in range(N_CORES):
        b, h = c // 2, c % 2
        rows = slice(h * LH, (h + 1) * LH)
        own = slice(h * FREE, (h + 1) * FREE)
        in_maps.append({
            "xn": np.ascontiguousarray(xb[b, rows, :]),
            "xT": np.ascontiguousarray(xb[b].T[:, rows]),
            "slots": np.array([[h, 1 - h]], dtype=np.uint32),
            "wq": wq_h, "wk": wk_h,
            "wvT": np.ascontiguousarray(wvT_f[:, own]),
        })

    nc = _get_nc()
    res = run_bass_kernel_spmd(nc, in_maps, list(range(N_CORES)), trace=trace)

    full = np.empty((B, L, D), dtype=np.float32)
    for c in range(N_CORES):
        b, h = c // 2, c % 2
        dev = res.results[c]["out"]
        rows = slice(h * LH, (h + 1) * LH)
        full[b, rows, h * FREE:(h + 1) * FREE] = dev[:, 0:FREE]
        full[b, rows, (1 - h) * FREE:(2 - h) * FREE] = dev[:, FREE:D]
    return full, res


def kernel(**inputs):
    full, _ = run(inputs, trace=False)
    return full
